# revision 9
# baseline (speedup 1.0000x reference)
"""DSVT cross-attention block on 8 Trainium2 NeuronCores.

Data-parallel over voxel rows: the voxel_inds gather/scatter is a pure
permutation (each voxel appears exactly once), so each core owns a
contiguous slice of gathered rows and runs the full per-row pipeline:

  q = (src+pos) @ WqT + bq                  (per-row)
  k/v from box features (tiny, replicated on every core)
  s = q @ k^T * scale + mask_bias           (mask folded into contraction)
  p = softmax_m(s)  (no max-subtraction; masked lanes get -3e4 bias)
  src2 = p @ (V @ Wo^T) + bo
  x  = LN1(src + src2);  out = LN2(x + FFN(x))

Layout strategy: activations row-major [rows, feat]; every matmul
contraction runs on partitions, so the row-side operand is transposed
on the fly with PE transposes (X, P, x_hat).  All biases/masks enter
matmuls through appended ones/vb contraction rows.  LN scale/bias are
applied via broadcast constant tiles; rsqrt = exp(-0.5*ln(v+eps)) keeps
every ScalarE op inside the natural_log_exp table set (one table load).
"""

import os
import sys

for _p in ("/opt/trn_rl_repo", os.path.expanduser("~/.axon_site/_ro/trn_rl_repo")):
    if os.path.isdir(_p) and _p not in sys.path:
        sys.path.append(_p)

import numpy as np

import concourse.bass as bass
import concourse.tile as tile
from concourse import bacc, mybir
from concourse.bass_utils import run_bass_kernel_spmd
from concourse.masks import make_identity

F32 = mybir.dt.float32
AF = mybir.ActivationFunctionType
ALU = mybir.AluOpType

N = 46080
D = 192
M = 256
BD = 256
H = 8
DH = 24
DFF = 2048
EPS = 1e-5
NCORES = 8
R = N // NCORES  # 5760 rows per core
P = 128
NT = R // P  # 45 row-tiles per core
SCALE = DH ** -0.5
BIG = 30000.0

_CACHE = {}


def build_nc(n_tiles=NT):
    """Build the per-core Bass program for n_tiles row-tiles of 128 rows."""
    rows = n_tiles * P
    nc = bacc.Bacc(None, target_bir_lowering=False)

    # ---- DRAM I/O ----------------------------------------------------
    srcr = nc.dram_tensor("srcr", [rows, D], F32, kind="ExternalInput")
    posr = nc.dram_tensor("posr", [rows, D], F32, kind="ExternalInput")
    vb1 = nc.dram_tensor("vb1", [rows, 2], F32, kind="ExternalInput")
    bf_t = nc.dram_tensor("bf_t", [BD, M], F32, kind="ExternalInput")
    bp_t = nc.dram_tensor("bp_t", [BD, M], F32, kind="ExternalInput")
    boxb = nc.dram_tensor("boxb", [1, M], F32, kind="ExternalInput")
    wq_l = nc.dram_tensor("wq_l", [D + 2, 256], F32, kind="ExternalInput")
    wk_l = nc.dram_tensor("wk_l", [BD + 2, 256], F32, kind="ExternalInput")
    wv_l = nc.dram_tensor("wv_l", [BD + 2, 256], F32, kind="ExternalInput")
    wo_r = nc.dram_tensor("wo_r", [256, D], F32, kind="ExternalInput")
    w1_l = nc.dram_tensor("w1_l", [D + 1, DFF], F32, kind="ExternalInput")
    w2_r = nc.dram_tensor("w2_r", [DFF, D], F32, kind="ExternalInput")
    cvec = nc.dram_tensor("cvec", [5, D], F32, kind="ExternalInput")
    outr = nc.dram_tensor("outr", [rows, D], F32, kind="ExternalOutput")

    with tile.TileContext(nc) as tc:
        with (
            tc.tile_pool(name="const", bufs=1) as const,
            tc.tile_pool(name="work", bufs=3) as work,
            tc.tile_pool(name="big", bufs=2) as bigp,
            tc.tile_pool(name="ps_s", bufs=4, space="PSUM") as ps_s,
            tc.tile_pool(name="ps_m", bufs=3, space="PSUM") as ps_m,
        ):
            # ---- constants / weights into SBUF -----------------------
            ident = const.tile([P, P], F32)
            make_identity(nc, ident)

            wq1 = const.tile([P, 256], F32)
            nc.sync.dma_start(wq1, wq_l[:P])
            wq2 = const.tile([66, 256], F32)
            nc.sync.dma_start(wq2, wq_l[P : D + 2])
            wk1 = const.tile([P, 256], F32)
            nc.sync.dma_start(wk1, wk_l[:P])
            wk2 = const.tile([P, 256], F32)
            nc.sync.dma_start(wk2, wk_l[P : 2 * P])
            wk3 = const.tile([2, 256], F32)
            nc.sync.dma_start(wk3, wk_l[2 * P :])
            wv1 = const.tile([P, 256], F32)
            nc.sync.dma_start(wv1, wv_l[:P])
            wv2 = const.tile([P, 256], F32)
            nc.sync.dma_start(wv2, wv_l[P : 2 * P])
            wv3 = const.tile([2, 256], F32)
            nc.sync.dma_start(wv3, wv_l[2 * P :])
            wo_sb = const.tile([64, 4, D], F32)
            nc.sync.dma_start(wo_sb, wo_r.rearrange("(g x) d -> x g d", g=4))
            w1a = const.tile([P, DFF], F32)
            nc.sync.dma_start(w1a, w1_l[:P])
            w1b = const.tile([65, DFF], F32)
            nc.sync.dma_start(w1b, w1_l[P : D + 1])
            w2sb = const.tile([P, 16, D], F32)
            nc.sync.dma_start(w2sb, w2_r.rearrange("(c p) d -> p c d", p=P))

            # broadcast constant rows -> [128, D] tiles
            g1b = const.tile([P, D], F32)
            nc.sync.dma_start(g1b, cvec[0:1, :].to_broadcast((P, D)))
            cb = const.tile([P, D], F32)
            nc.sync.dma_start(cb, cvec[1:2, :].to_broadcast((P, D)))
            g2b = const.tile([P, D], F32)
            nc.sync.dma_start(g2b, cvec[2:3, :].to_broadcast((P, D)))
            b2b = const.tile([P, D], F32)
            nc.sync.dma_start(b2b, cvec[3:4, :].to_broadcast((P, D)))
            bob = const.tile([P, D], F32)
            nc.sync.dma_start(bob, cvec[4:5, :].to_broadcast((P, D)))
            epst = const.tile([P, 1], F32)
            nc.vector.memset(epst, EPS)

            # ---- box-side precompute (tiny, once) --------------------
            kb1 = const.tile([P, M], F32)
            kb2 = const.tile([P, M], F32)
            kb3 = const.tile([2, M], F32)
            vb2t = const.tile([P, M], F32)  # bf_t rows 128..255 (V path)
            nc.sync.dma_start(vb2t, bf_t[P:])
            bp1 = work.tile([P, M], F32, name="bp1")
            nc.sync.dma_start(bp1, bp_t[:P])
            bp2 = work.tile([P, M], F32, name="bp2")
            nc.sync.dma_start(bp2, bp_t[P:])
            vb1t = const.tile([P, M], F32)  # bf_t rows 0..127
            nc.sync.dma_start(vb1t, bf_t[:P])
            nc.vector.tensor_add(kb1, vb1t, bp1)
            nc.vector.tensor_add(kb2, vb2t, bp2)
            nc.vector.memset(kb3[0:1, :], 1.0)
            nc.sync.dma_start(kb3[1:2, :], boxb[:])
            vb3 = const.tile([2, M], F32)
            nc.vector.memset(vb3, 0.0)
            nc.vector.memset(vb3[0:1, :], 1.0)

            # KE[g] : [104, 256] scores-side keys (+mask rows)
            ke = []
            for g in range(4):
                ps_ke = ps_m.tile([64, M], F32, name=f"ps_ke{g}", tag="psm")
                cs = slice(64 * g, 64 * (g + 1))
                nc.tensor.matmul(ps_ke, wk1[:, cs], kb1, start=True, stop=False)
                nc.tensor.matmul(ps_ke, wk2[:, cs], kb2, start=False, stop=False)
                nc.tensor.matmul(ps_ke, wk3[:, cs], kb3, start=False, stop=True)
                ke_g = const.tile([64, M], F32, name=f"ke{g}")
                nc.scalar.copy(ke_g, ps_ke)
                ke.append(ke_g)

            # VT[g] : [96, 256] value rows grouped per head
            vt = []
            for g in range(4):
                ps_vt = ps_m.tile([64, M], F32, name=f"ps_vt{g}", tag="psm")
                cs = slice(64 * g, 64 * (g + 1))
                nc.tensor.matmul(ps_vt, wv1[:, cs], vb1t, start=True, stop=False)
                nc.tensor.matmul(ps_vt, wv2[:, cs], vb2t, start=False, stop=False)
                nc.tensor.matmul(ps_vt, wv3[:, cs], vb3, start=False, stop=True)
                vt_g = const.tile([64, M], F32, name=f"vt{g}")
                nc.scalar.copy(vt_g, ps_vt)
                vt.append(vt_g)

            # B = concat_h V_h @ Wo_h^T : [2048, 192] as [128, 16, 192]
            bmat = const.tile([P, 16, D], F32)
            for h in range(H):
                g, hp = divmod(h, 2)
                for mh in range(2):
                    ps_b = ps_m.tile([P, D], F32, name="ps_b", tag="psm")
                    nc.tensor.matmul(
                        ps_b,
                        vt[g][32 * hp : 32 * hp + 24, mh * P : (mh + 1) * P],
                        wo_sb[32 * hp : 32 * hp + 24, g, :],
                        start=True,
                        stop=True,
                    )
                    nc.scalar.copy(bmat[:, 2 * h + mh, :], ps_b)

            # ---- main loop over row-tiles ----------------------------
            for t in range(n_tiles):
                rs = slice(t * P, (t + 1) * P)
                srct = work.tile([P, D], F32, name="srct")
                nc.sync.dma_start(srct, srcr[rs])
                xt = work.tile([P, D + 2], F32, name="xt")
                nc.sync.dma_start(xt[:, :D], posr[rs])
                nc.sync.dma_start(xt[:, D : D + 2], vb1[rs])
                nc.vector.tensor_add(xt[:, :D], xt[:, :D], srct)

                # X_ext^T
                ps_x1 = ps_s.tile([P, P], F32, name="ps_x1", tag="ps")
                nc.tensor.transpose(ps_x1, xt[:, :P], ident)
                ps_x2 = ps_s.tile([66, P], F32, name="ps_x2", tag="ps")
                nc.tensor.transpose(ps_x2, xt[:, P : D + 2], ident)
                xt1 = work.tile([P, P], F32, name="xt1")
                nc.scalar.copy(xt1, ps_x1)
                xt2 = work.tile([66, P], F32, name="xt2")
                nc.scalar.copy(xt2, ps_x2)

                # Q_ext^T (per 4-head group) : [104, 128]
                qe = []
                for g in range(4):
                    cs = slice(64 * g, 64 * (g + 1))
                    ps_q = ps_s.tile([64, P], F32, name=f"ps_q{g}", tag="ps")
                    nc.tensor.matmul(ps_q, wq1[:, cs], xt1, start=True, stop=False)
                    nc.tensor.matmul(ps_q, wq2[:, cs], xt2, start=False, stop=True)
                    qe_g = work.tile([64, P], F32, name=f"qe{g}")
                    nc.scalar.copy(qe_g, ps_q)
                    qe.append(qe_g)

                # scores -> exp -> row-sums
                pt_sb = bigp.tile([P, H, M], F32, name="pt_sb")
                ssum = work.tile([P, H], F32, name="ssum")
                for h in range(H):
                    g, hp = divmod(h, 2)
                    hs = slice(32 * hp, 32 * hp + 26)
                    ps_sc = ps_s.tile([P, M], F32, name="ps_sc", tag="ps")
                    nc.tensor.matmul(
                        ps_sc, qe[g][hs, :], ke[g][hs, :], start=True, stop=True
                    )
                    nc.scalar.activation(
                        pt_sb[:, h, :],
                        ps_sc,
                        AF.Exp,
                        accum_out=ssum[:, h : h + 1],
                    )
                rsum = work.tile([P, H], F32, name="rsum")
                nc.vector.reciprocal(rsum, ssum)
                for h in range(H):
                    nc.vector.tensor_scalar_mul(
                        pt_sb[:, h, :], pt_sb[:, h, :], rsum[:, h : h + 1]
                    )

                # transpose P -> [m, r] chunks, then src2 = P @ B
                ptr = bigp.tile([P, 16, P], F32, name="ptr")
                for c in range(16):
                    h, mh = divmod(c, 2)
                    ps_pt = ps_s.tile([P, P], F32, name="ps_pt", tag="ps")
                    nc.tensor.transpose(
                        ps_pt, pt_sb[:, h, mh * P : (mh + 1) * P], ident
                    )
                    if c % 2 == 0:
                        nc.scalar.copy(ptr[:, c, :], ps_pt)
                    else:
                        nc.vector.tensor_copy(ptr[:, c, :], ps_pt)

                ps_av = ps_m.tile([P, D], F32, name="ps_av", tag="psm")
                for c in range(16):
                    nc.tensor.matmul(
                        ps_av,
                        ptr[:, c, :],
                        bmat[:, c, :],
                        start=(c == 0),
                        stop=(c == 15),
                    )

                # LN1
                a1 = work.tile([P, D], F32, name="a1")
                nc.vector.tensor_add(a1, ps_av, bob)
                nc.vector.tensor_add(a1, a1, srct)
                st1 = work.tile([P, 6], F32, name="st1")
                nc.vector.bn_stats(st1, a1)
                mv1 = work.tile([P, 2], F32, name="mv1")
                nc.vector.bn_aggr(mv1, st1)
                lnv = work.tile([P, 2], F32, name="lnv")
                nc.scalar.activation(lnv[:, 0:1], mv1[:, 1:2], AF.Ln, bias=epst)
                nc.scalar.activation(lnv[:, 1:2], lnv[:, 0:1], AF.Exp, scale=-0.5)
                xh = work.tile([P, D + 1], F32, name="xh")
                nc.vector.tensor_scalar(
                    xh[:, :D],
                    a1,
                    mv1[:, 0:1],
                    lnv[:, 1:2],
                    op0=ALU.subtract,
                    op1=ALU.mult,
                )
                nc.vector.memset(xh[:, D : D + 1], 1.0)

                # x_hat^T
                ps_h1 = ps_s.tile([P, P], F32, name="ps_h1", tag="ps")
                nc.tensor.transpose(ps_h1, xh[:, :P], ident)
                ps_h2 = ps_s.tile([65, P], F32, name="ps_h2", tag="ps")
                nc.tensor.transpose(ps_h2, xh[:, P : D + 1], ident)
                xh1 = work.tile([P, P], F32, name="xh1")
                nc.scalar.copy(xh1, ps_h1)
                xh2 = work.tile([65, P], F32, name="xh2")
                nc.scalar.copy(xh2, ps_h2)

                # FFN: h^T chunks with fused relu, then z = relu(h)^T.T @ W2^T
                ht = bigp.tile([P, 16, P], F32, name="ht")
                for c in range(16):
                    cs = slice(c * P, (c + 1) * P)
                    ps_f = ps_s.tile([P, P], F32, name="ps_f", tag="ps")
                    nc.tensor.matmul(ps_f, w1a[:, cs], xh1, start=True, stop=False)
                    nc.tensor.matmul(ps_f, w1b[:, cs], xh2, start=False, stop=True)
                    if c % 2 == 0:
                        nc.scalar.activation(ht[:, c, :], ps_f, AF.Relu)
                    else:
                        nc.vector.tensor_relu(ht[:, c, :], ps_f)

                ps_z = ps_m.tile([P, D], F32, name="ps_z", tag="psm")
                for c in range(16):
                    nc.tensor.matmul(
                        ps_z,
                        ht[:, c, :],
                        w2sb[:, c, :],
                        start=(c == 0),
                        stop=(c == 15),
                    )

                # z = xh*g1 + ffn + (ln1_b + b2)
                zt = work.tile([P, D], F32, name="zt")
                nc.vector.tensor_mul(zt, xh[:, :D], g1b)
                nc.vector.tensor_add(zt, zt, ps_z)
                nc.vector.tensor_add(zt, zt, cb)

                # LN2 + affine
                st2 = work.tile([P, 6], F32, name="st2")
                nc.vector.bn_stats(st2, zt)
                mv2 = work.tile([P, 2], F32, name="mv2")
                nc.vector.bn_aggr(mv2, st2)
                lnv2 = work.tile([P, 2], F32, name="lnv2")
                nc.scalar.activation(lnv2[:, 0:1], mv2[:, 1:2], AF.Ln, bias=epst)
                nc.scalar.activation(lnv2[:, 1:2], lnv2[:, 0:1], AF.Exp, scale=-0.5)
                ot = work.tile([P, D], F32, name="ot")
                nc.vector.tensor_scalar(
                    ot,
                    zt,
                    mv2[:, 0:1],
                    lnv2[:, 1:2],
                    op0=ALU.subtract,
                    op1=ALU.mult,
                )
                nc.vector.tensor_mul(ot, ot, g2b)
                nc.vector.tensor_add(ot, ot, b2b)
                nc.sync.dma_start(outr[rs], ot)

    nc.finalize()
    return nc


def prep_consts(inputs):
    """Host-side weight/constant reformatting (layout only, plus the
    standard scale/LN-gamma folds into static weights)."""
    f = np.float32
    Wq = np.asarray(inputs["Wq"], f)
    bq = np.asarray(inputs["bq"], f)
    Wk = np.asarray(inputs["Wk"], f)
    bk = np.asarray(inputs["bk"], f)
    Wv = np.asarray(inputs["Wv"], f)
    bv = np.asarray(inputs["bv"], f)
    Wo = np.asarray(inputs["Wo"], f)
    bo = np.asarray(inputs["bo"], f)
    W1 = np.asarray(inputs["W1"], f)
    b1 = np.asarray(inputs["b1"], f)
    W2 = np.asarray(inputs["W2"], f)
    b2 = np.asarray(inputs["b2"], f)
    g1 = np.asarray(inputs["ln1_g"], f)
    lb1 = np.asarray(inputs["ln1_b"], f)
    g2 = np.asarray(inputs["ln2_g"], f)
    lb2 = np.asarray(inputs["ln2_b"], f)

    Wq_s = Wq * SCALE
    bq_s = bq * SCALE

    wq_l = np.zeros((D + 2, 256), f)
    wk_l = np.zeros((BD + 2, 256), f)
    wv_l = np.zeros((BD + 2, 256), f)
    wo_r = np.zeros((256, D), f)
    for h in range(H):
        c0 = 32 * h
        hd = slice(DH * h, DH * (h + 1))
        wq_l[:D, c0 : c0 + DH] = Wq_s[hd].T
        wq_l[D + 1, c0 : c0 + DH] = bq_s[hd]
        wq_l[D, c0 + 24] = 1.0  # vb passthrough
        wq_l[D + 1, c0 + 25] = 1.0  # ones passthrough
        wk_l[:BD, c0 : c0 + DH] = Wk[hd].T
        wk_l[BD, c0 : c0 + DH] = bk[hd]
        wk_l[BD, c0 + 24] = -BIG
        wk_l[BD + 1, c0 + 24] = 2.0 * BIG
        wk_l[BD + 1, c0 + 25] = -BIG
        wv_l[:BD, c0 : c0 + DH] = Wv[hd].T
        wv_l[BD, c0 : c0 + DH] = bv[hd]
        wo_r[c0 : c0 + DH, :] = Wo[:, hd].T
    w1_l = np.zeros((D + 1, DFF), f)
    w1_l[:D] = (W1 * g1[None, :]).T
    w1_l[D] = b1 + W1 @ lb1
    w2_r = np.ascontiguousarray(W2.T)
    cvec = np.stack([g1, lb1 + b2, g2, lb2, bo]).astype(f)

    bf_t = np.ascontiguousarray(np.asarray(inputs["box_feature"], f).T)
    bp_t = np.ascontiguousarray(np.asarray(inputs["box_pos"], f).T)
    boxb = np.asarray(inputs["box_voxel_coords"])[:, 0].astype(f)[None, :]
    boxb = np.ascontiguousarray(boxb)

    return {
        "bf_t": bf_t,
        "bp_t": bp_t,
        "boxb": boxb,
        "wq_l": wq_l,
        "wk_l": wk_l,
        "wv_l": wv_l,
        "wo_r": wo_r,
        "w1_l": w1_l,
        "w2_r": w2_r,
        "cvec": cvec,
    }


def kernel(**inputs):
    src = np.asarray(inputs["src"], np.float32)
    pos = np.asarray(inputs["pos"], np.float32)
    vinds = np.asarray(inputs["voxel_inds"]).reshape(-1).astype(np.int64)
    vcoords = np.asarray(inputs["voxel_coords"])

    perm = vinds  # gather order; it is a permutation of range(N)
    src_g = np.ascontiguousarray(src[perm])
    pos_g = np.ascontiguousarray(pos[perm])
    vb = vcoords[perm, 0].astype(np.float32)
    vb1 = np.stack([vb, np.ones_like(vb)], axis=1)
    vb1 = np.ascontiguousarray(vb1)

    consts = prep_consts(inputs)

    if "nc" not in _CACHE:
        _CACHE["nc"] = build_nc(NT)
    nc = _CACHE["nc"]

    in_maps = []
    for c in range(NCORES):
        rs = slice(c * R, (c + 1) * R)
        m = dict(consts)
        m["srcr"] = src_g[rs]
        m["posr"] = pos_g[rs]
        m["vb1"] = vb1[rs]
        in_maps.append(m)

    res = run_bass_kernel_spmd(nc, in_maps, list(range(NCORES)))
    out_rows = np.concatenate([res.results[c]["outr"] for c in range(NCORES)], axis=0)

    out = np.empty((N, D), np.float32)
    out[perm] = out_rows
    return out
